# revision 6
# baseline (speedup 1.0000x reference)
"""Trainium2 Bass kernel for LittleBitLinear reconstruction (fp8 DoubleRow).

Computes M = (sign(U_fp) * ell) @ sign(V_fp)^T * g[None, :] * h[:, None]
for U_fp (4096, 1024), V_fp (11008, 1024) -> M (4096, 11008) fp32.

Strategy: shard d_in (rows of V_fp / columns of M) across 8 cores; U_fp, h,
ell replicated. Each core computes the full 4096 x 1376 column block.

Key idea: the matmul operands are pure signs scaled per contraction index r
by ell[r]. Factor |ell[r]| ~= alpha[r] * beta[r] with both factors exactly on
the fp8-e4m3 grid (error ~0.9% rms, deterministic), fold sign(ell) into
beta. Then A[r, m] = sign(U)*alpha[r] and B[r, n] = sign(V)*sign(ell)*beta[r]
are EXACT fp8 values, and the fp8 DoubleRow matmul (2x bf16 throughput,
256-deep contraction per pass, fp32 PSUM accumulation) computes
sum_r sign(U)*sign(V)*alpha*beta exactly up to fp32 accumulation. g and h are
applied exactly at PSUM evacuation (ACT per-partition scale for h, DVE
elementwise for g), so the dominant approximation is ell -> alpha*beta.

Staging needs no Sign activation: host ships U^T / V^T as raw fp8 bytes
(only the sign bit is consumed -- cast preserves it for every value incl.
+-0). The scale factor alpha[r]/beta[r] depends only on the SBUF partition,
so adjacent byte pairs share it and staging runs as uint16:
(bytes16 & 0x8080) ^ (alpha<<8|alpha) = sign*scale for two fp8 lanes at once,
hitting the DVE 16-bit fast path.
"""

import os
import sys

import numpy as np

for _p in ("/opt/trn_rl_repo",):
    if _p not in sys.path and os.path.isdir(_p):
        sys.path.insert(0, _p)

D_OUT, D_IN, R, NCORES = 4096, 11008, 1024, 8
N_SH = D_IN // NCORES  # 1376
P = 128
KB = R // P            # 8 k-blocks
KPAIR = KB // 2        # 4 double-row pairs
OB = D_OUT // P        # 32 o-blocks


def _n_tiles(n_sh, max_n=512):
    # narrow tile first: the next (j,s) LDWEIGHTS hides best under a
    # trailing full-width stream
    tiles = []
    c0 = 0
    while c0 < n_sh:
        nw = min(max_n, n_sh - c0)
        tiles.append((c0, nw))
        c0 += nw
    return tiles[::-1]


def build_program(d_out=D_OUT, n_sh=N_SH, r=R, reps=1, skip=(), max_n=512,
                  psum_cols=1536):
    """Build the per-core Bass program (SPMD: same program, different data)."""
    from contextlib import ExitStack

    import concourse.bass as bass  # noqa: F401
    import concourse.mybir as mybir
    import concourse.tile as tile
    from concourse import bacc

    f32 = mybir.dt.float32
    bf16 = mybir.dt.bfloat16
    u8 = mybir.dt.uint8
    u16 = mybir.dt.uint16
    fp8 = mybir.dt.float8e4
    AF = mybir.ActivationFunctionType
    ALU = mybir.AluOpType
    DR = mybir.MatmulPerfMode.DoubleRow

    kblocks = r // P
    kpairs = kblocks // 2
    oblocks = d_out // P
    ntiles = _n_tiles(n_sh, max_n=max_n)

    nc = bacc.Bacc(None, target_bir_lowering=False)
    ut = nc.declare_dram_parameter("ut", [r, d_out], fp8, isOutput=False)
    vt = nc.declare_dram_parameter("vt", [r, n_sh], fp8, isOutput=False)
    ab = nc.declare_dram_parameter("ab", [P, kblocks], u16, isOutput=False)
    bb = nc.declare_dram_parameter("bb", [P, kblocks], u16, isOutput=False)
    hh = nc.declare_dram_parameter("h", [P, oblocks], f32, isOutput=False)
    gg = nc.declare_dram_parameter("g", [P, n_sh], bf16, isOutput=False)
    out = nc.declare_dram_parameter("out", [d_out, n_sh], bf16, isOutput=True)

    with tile.TileContext(nc) as tc, ExitStack() as ctx:
        consts = ctx.enter_context(tc.tile_pool(name="consts", bufs=1))
        ustg = ctx.enter_context(tc.tile_pool(name="ustg", bufs=3))
        vstg = ctx.enter_context(tc.tile_pool(name="vstg", bufs=3))
        abuf = ctx.enter_context(tc.tile_pool(name="abuf", bufs=1))
        bbuf = ctx.enter_context(tc.tile_pool(name="bbuf", bufs=1))
        outp = ctx.enter_context(tc.tile_pool(name="outp", bufs=4))
        outp2 = ctx.enter_context(tc.tile_pool(name="outp2", bufs=4))
        psum = ctx.enter_context(tc.tile_pool(name="psum", bufs=2, space="PSUM"))

        # Route shared operands through one ACT copy so downstream ops carry
        # a single cross-proc wait.
        ab_raw = consts.tile([P, kblocks], u16)
        nc.sync.dma_start(out=ab_raw, in_=ab[:, :])
        ab_sb = consts.tile([P, kblocks], u16)
        nc.scalar.activation(out=ab_sb, in_=ab_raw, func=AF.Copy)
        bb_raw = consts.tile([P, kblocks], u16)
        nc.sync.dma_start(out=bb_raw, in_=bb[:, :])
        bb_sb = consts.tile([P, kblocks], u16)
        nc.scalar.activation(out=bb_sb, in_=bb_raw, func=AF.Copy)
        h_raw = consts.tile([P, oblocks], f32)
        nc.sync.dma_start(out=h_raw, in_=hh[:, :])
        h_sb = consts.tile([P, oblocks], f32)
        nc.scalar.activation(out=h_sb, in_=h_raw, func=AF.Copy)
        g_raw = consts.tile([P, n_sh], bf16)
        nc.sync.dma_start(out=g_raw, in_=gg[:, :])
        g_sb = consts.tile([P, n_sh], bf16)
        nc.scalar.activation(out=g_sb, in_=g_raw, func=AF.Copy)

        for rep in range(reps):
            # A: [128, KB, d_out] fp8, B: [128, KB, n_sh] fp8 -- pair dim in
            # the middle so DoubleRow can slice [:, 2s:2s+2, cols].
            at = abuf.tile([P, kblocks, d_out], fp8, tag="a", name=f"at_{rep}")
            bt = bbuf.tile([P, kblocks, n_sh], fp8, tag="b", name=f"bt_{rep}")
            at16 = at.bitcast(u16)
            bt16 = bt.bitcast(u16)

            if "stage" not in skip:
                # spread input DMAs across three rings (sync/gpsimd/scalar)
                # so dispatch serialization and wire time overlap
                for k in range(kblocks):
                    vst = vstg.tile([P, n_sh // 2], u16, tag="vstg",
                                    name=f"vst_{rep}_{k}")
                    nc.gpsimd.dma_start(
                        out=vst, in_=vt[k * P:(k + 1) * P, :].bitcast(u16)
                    )
                    nc.vector.tensor_scalar(
                        out=bt16[:, k, :], in0=vst, scalar1=0x8080,
                        scalar2=bb_sb[:, k:k + 1],
                        op0=ALU.bitwise_and, op1=ALU.bitwise_xor,
                    )
                    ust = ustg.tile([P, d_out // 2], u16, tag="ustg",
                                    name=f"ust_{rep}_{k}")
                    ueng = nc.sync if k % 2 == 0 else nc.scalar
                    ueng.dma_start(
                        out=ust, in_=ut[k * P:(k + 1) * P, :].bitcast(u16)
                    )
                    nc.vector.tensor_scalar(
                        out=at16[:, k, :], in0=ust, scalar1=0x8080,
                        scalar2=ab_sb[:, k:k + 1],
                        op0=ALU.bitwise_and, op1=ALU.bitwise_xor,
                    )
            else:
                nc.vector.memset(at16[:, :, 0:1], 0x3030)
                nc.vector.memset(bt16[:, :, 0:1], 0x3030)

            # --- DoubleRow matmuls + evacuate
            for j in range(oblocks):
                pt = psum.tile([P, psum_cols], f32, tag="ps", name=f"ps_{rep}_{j}")
                if "mm" not in skip:
                    for s in range(kpairs):
                        lhsT = at[:, 2 * s:2 * s + 2, j * P:(j + 1) * P]
                        for (c0, nw) in ntiles:
                            nc.tensor.matmul(
                                pt[:, c0:c0 + nw], lhsT=lhsT,
                                rhs=bt[:, 2 * s:2 * s + 2, c0:c0 + nw],
                                start=(s == 0), stop=(s == kpairs - 1),
                                perf_mode=DR,
                            )
                ot = outp.tile([P, n_sh], bf16, tag="out", name=f"ot_{rep}_{j}")
                ot2 = outp2.tile([P, n_sh], bf16, tag="out2", name=f"ot2_{rep}_{j}")
                if "evac" not in skip:
                    # h via ACT per-partition scale (PSUM f32 -> SBUF bf16),
                    # then g via DVE bf16 tensor_tensor (2x mode)
                    nc.scalar.activation(
                        out=ot, in_=pt[:, 0:n_sh], func=AF.Copy,
                        scale=h_sb[:, j:j + 1],
                    )
                    nc.vector.tensor_tensor(
                        out=ot2, in0=ot, in1=g_sb, op=ALU.mult,
                    )
                else:
                    nc.vector.memset(ot2[:, 0:1], 0.0)
                if "outdma" not in skip:
                    nc.gpsimd.dma_start(out=out[j * P:(j + 1) * P, :], in_=ot2)

    nc.compile()
    return nc


_NC_CACHE = {}


def _get_nc():
    if "nc" not in _NC_CACHE:
        _NC_CACHE["nc"] = build_program()
    return _NC_CACHE["nc"]


def _e4m3_normal_grid():
    import ml_dtypes

    vals = []
    for bits in range(1, 0x7F):
        f = float(np.uint8(bits).view(ml_dtypes.float8_e4m3fn))
        if np.isfinite(f) and 0.015625 <= f <= 240.0:
            vals.append(f)
    return np.array(sorted(set(vals)))


def _factorize_ell(ell):
    """Best alpha*beta ~= |ell| with both factors on the normal e4m3 grid.

    Balanced around sqrt|ell| so neither factor goes subnormal. Returns
    (alpha_f32 (>0), beta_signed_f32) with alpha * beta_signed ~= ell.
    """
    grid = _e4m3_normal_grid()
    a_ell = np.abs(ell).astype(np.float64)
    sq = np.sqrt(a_ell)
    ai = np.searchsorted(grid, sq)
    best_a = np.ones_like(a_ell)
    best_b = np.ones_like(a_ell)
    best_err = np.full_like(a_ell, np.inf)
    for off in range(-24, 25):
        idx = np.clip(ai + off, 0, len(grid) - 1)
        alpha = grid[idx]
        tgt = a_ell / alpha
        bi = np.searchsorted(grid, tgt)
        for boff in (-1, 0):
            bidx = np.clip(bi + boff, 0, len(grid) - 1)
            beta = grid[bidx]
            err = np.abs(alpha * beta - a_ell)
            take = err < best_err
            best_a = np.where(take, alpha, best_a)
            best_b = np.where(take, beta, best_b)
            best_err = np.where(take, err, best_err)
    return (
        best_a.astype(np.float32),
        (best_b * np.sign(ell)).astype(np.float32),
    )


def _make_in_maps(U_fp, V_fp, h, g, ell):
    import ml_dtypes

    FP8 = ml_dtypes.float8_e4m3fn

    U_fp = np.ascontiguousarray(np.asarray(U_fp, dtype=np.float32))
    V_fp = np.ascontiguousarray(np.asarray(V_fp, dtype=np.float32))
    h = np.asarray(h, dtype=np.float32).reshape(-1)
    g = np.asarray(g, dtype=np.float32).reshape(-1)
    ell = np.asarray(ell, dtype=np.float32).reshape(-1)

    alpha, beta_s = _factorize_ell(ell)

    # fp8 byte planes: only the sign bit of ut/vt is consumed on device.
    # Scale bytes are doubled into u16 (two fp8 lanes share the partition's
    # scale) for the packed staging op.
    ut = np.ascontiguousarray(U_fp.T).astype(FP8)            # (R, D_OUT)

    def dbl(x):  # fp8 byte -> 0xBBBB u16
        b = x.astype(FP8).view(np.uint8).astype(np.uint16)
        return (b | (b << 8)).reshape(KB, P).T.copy()        # (128, KB)

    ab16 = dbl(alpha)
    bb16 = dbl(beta_s)
    h_t = np.ascontiguousarray(h.reshape(OB, P).T)           # (128, 32)

    in_maps = []
    for c in range(NCORES):
        sl = slice(c * N_SH, (c + 1) * N_SH)
        in_maps.append({
            "ut": ut,
            "vt": np.ascontiguousarray(V_fp[sl, :].T).astype(FP8),  # (R, N_SH)
            "ab": ab16,
            "bb": bb16,
            "h": h_t,
            "g": np.ascontiguousarray(
                np.broadcast_to(g[sl].reshape(1, N_SH), (P, N_SH))
            ).astype(ml_dtypes.bfloat16),
        })
    return in_maps


def run(U_fp, V_fp, h, g, ell, trace=False):
    """Run on 8 NeuronCores; returns (M, BassKernelResults)."""
    from concourse.bass_utils import run_bass_kernel_spmd

    nc = _get_nc()
    in_maps = _make_in_maps(U_fp, V_fp, h, g, ell)
    res = run_bass_kernel_spmd(nc, in_maps, list(range(NCORES)), trace=trace)
    M = np.concatenate(
        [res.results[c]["out"].astype(np.float32) for c in range(NCORES)],
        axis=1,
    )
    return M, res


def kernel(U_fp, V_fp, h, g, ell):
    M, _ = run(U_fp, V_fp, h, g, ell, trace=False)
    return M


# revision 8
# speedup vs baseline: 1.0381x; 1.0381x over previous
"""Trainium2 Bass kernel for LittleBitLinear reconstruction (fp8 DoubleRow).

Computes M = (sign(U_fp) * ell) @ sign(V_fp)^T * g[None, :] * h[:, None]
for U_fp (4096, 1024), V_fp (11008, 1024) -> M (4096, 11008) fp32.

Strategy: shard d_in (rows of V_fp / columns of M) across 8 cores; U_fp, h,
ell replicated. Each core computes the full 4096 x 1376 column block.

Key idea: the matmul operands are pure signs scaled per contraction index r
by ell[r]. Factor |ell[r]| ~= alpha[r] * beta[r] with both factors exactly on
the fp8-e4m3 grid (error ~0.9% rms, deterministic), fold sign(ell) into
beta. Then A[r, m] = sign(U)*alpha[r] and B[r, n] = sign(V)*sign(ell)*beta[r]
are EXACT fp8 values, and the fp8 DoubleRow matmul (2x bf16 throughput,
256-deep contraction per pass, fp32 PSUM accumulation) computes
sum_r sign(U)*sign(V)*alpha*beta exactly up to fp32 accumulation. g and h are
applied exactly at PSUM evacuation (ACT per-partition scale for h, DVE
elementwise for g), so the dominant approximation is ell -> alpha*beta.

Staging needs no Sign activation: host ships U^T / V^T as raw fp8 bytes
(only the sign bit is consumed -- cast preserves it for every value incl.
+-0). The scale factor alpha[r]/beta[r] depends only on the SBUF partition,
so adjacent byte pairs share it and staging runs as uint16:
(bytes16 & 0x8080) ^ (alpha<<8|alpha) = sign*scale for two fp8 lanes at once,
hitting the DVE 16-bit fast path.
"""

import os
import sys

import numpy as np

for _p in ("/opt/trn_rl_repo",):
    if _p not in sys.path and os.path.isdir(_p):
        sys.path.insert(0, _p)

D_OUT, D_IN, R, NCORES = 4096, 11008, 1024, 8
N_SH = D_IN // NCORES  # 1376
P = 128
KB = R // P            # 8 k-blocks
KPAIR = KB // 2        # 4 double-row pairs
OB = D_OUT // P        # 32 o-blocks


def _n_tiles(n_sh, max_n=512):
    # narrow tile first: the next (j,s) LDWEIGHTS hides best under a
    # trailing full-width stream
    tiles = []
    c0 = 0
    while c0 < n_sh:
        nw = min(max_n, n_sh - c0)
        tiles.append((c0, nw))
        c0 += nw
    return tiles[::-1]


def build_program(d_out=D_OUT, n_sh=N_SH, r=R, reps=1, skip=(), max_n=512,
                  psum_cols=1536):
    """Build the per-core Bass program (SPMD: same program, different data)."""
    from contextlib import ExitStack

    import concourse.bass as bass  # noqa: F401
    import concourse.mybir as mybir
    import concourse.tile as tile
    from concourse import bacc

    f32 = mybir.dt.float32
    bf16 = mybir.dt.bfloat16
    u8 = mybir.dt.uint8
    u16 = mybir.dt.uint16
    fp8 = mybir.dt.float8e4
    AF = mybir.ActivationFunctionType
    ALU = mybir.AluOpType
    DR = mybir.MatmulPerfMode.DoubleRow

    kblocks = r // P
    kpairs = kblocks // 2
    oblocks = d_out // P
    ntiles = _n_tiles(n_sh, max_n=max_n)

    nc = bacc.Bacc(None, target_bir_lowering=False)
    ut = nc.declare_dram_parameter("ut", [r, d_out], fp8, isOutput=False)
    vt = nc.declare_dram_parameter("vt", [r, n_sh], fp8, isOutput=False)
    ab = nc.declare_dram_parameter("ab", [P, kblocks], u16, isOutput=False)
    bb = nc.declare_dram_parameter("bb", [P, kblocks], u16, isOutput=False)
    hh = nc.declare_dram_parameter("h", [P, oblocks], f32, isOutput=False)
    gg = nc.declare_dram_parameter("g", [P, n_sh], bf16, isOutput=False)
    out = nc.declare_dram_parameter("out", [d_out, n_sh], bf16, isOutput=True)

    with tile.TileContext(nc) as tc, ExitStack() as ctx:
        consts = ctx.enter_context(tc.tile_pool(name="consts", bufs=1))
        ustg = ctx.enter_context(tc.tile_pool(name="ustg", bufs=3))
        vstg = ctx.enter_context(tc.tile_pool(name="vstg", bufs=3))
        abuf = ctx.enter_context(tc.tile_pool(name="abuf", bufs=1))
        bbuf = ctx.enter_context(tc.tile_pool(name="bbuf", bufs=1))
        outp = ctx.enter_context(tc.tile_pool(name="outp", bufs=4))
        outp2 = ctx.enter_context(tc.tile_pool(name="outp2", bufs=4))
        psum = ctx.enter_context(tc.tile_pool(name="psum", bufs=2, space="PSUM"))

        # Consts arrive via the scalar ring (idle early; keeps the sync ring
        # free for ut/vt). DVE copies collapse their sync source so staging
        # TS (DVE) and evac TT (DVE) see an engine-internal dep for the
        # scale operands -- a single cross-proc wait remains (DMA / ACT).
        ab_raw = consts.tile([P, kblocks], u16)
        nc.scalar.dma_start(out=ab_raw, in_=ab[:, :])
        ab_sb = consts.tile([P, kblocks], u16)
        nc.vector.tensor_scalar(out=ab_sb, in0=ab_raw, scalar1=0,
                                op0=ALU.bitwise_or, scalar2=None)
        bb_raw = consts.tile([P, kblocks], u16)
        nc.scalar.dma_start(out=bb_raw, in_=bb[:, :])
        bb_sb = consts.tile([P, kblocks], u16)
        nc.vector.tensor_scalar(out=bb_sb, in0=bb_raw, scalar1=0,
                                op0=ALU.bitwise_or, scalar2=None)
        h_raw = consts.tile([P, oblocks], f32)
        nc.scalar.dma_start(out=h_raw, in_=hh[:, :])
        h_sb = consts.tile([P, oblocks], f32)
        nc.scalar.activation(out=h_sb, in_=h_raw, func=AF.Copy)
        g_raw = consts.tile([P, n_sh], bf16)
        nc.scalar.dma_start(out=g_raw, in_=gg[:, :])
        g_sb = consts.tile([P, n_sh], bf16)
        nc.vector.tensor_scalar(out=g_sb, in0=g_raw, scalar1=0.0,
                                op0=ALU.add, scalar2=None)

        for rep in range(reps):
            # A: [128, KB, d_out] fp8, B: [128, KB, n_sh] fp8 -- pair dim in
            # the middle so DoubleRow can slice [:, 2s:2s+2, cols].
            at = abuf.tile([P, kblocks, d_out], fp8, tag="a", name=f"at_{rep}")
            bt = bbuf.tile([P, kblocks, n_sh], fp8, tag="b", name=f"bt_{rep}")
            at16 = at.bitcast(u16)
            bt16 = bt.bitcast(u16)

            if "stage" not in skip:
                # spread input DMAs across three rings (sync/gpsimd/scalar)
                # so dispatch serialization and wire time overlap
                for k in range(kblocks):
                    vst = vstg.tile([P, n_sh // 2], u16, tag="vstg",
                                    name=f"vst_{rep}_{k}")
                    nc.sync.dma_start(
                        out=vst, in_=vt[k * P:(k + 1) * P, :].bitcast(u16)
                    )
                    nc.vector.tensor_scalar(
                        out=bt16[:, k, :], in0=vst, scalar1=0x8080,
                        scalar2=bb_sb[:, k:k + 1],
                        op0=ALU.bitwise_and, op1=ALU.bitwise_xor,
                    )
                    ust = ustg.tile([P, d_out // 2], u16, tag="ustg",
                                    name=f"ust_{rep}_{k}")
                    nc.sync.dma_start(
                        out=ust, in_=ut[k * P:(k + 1) * P, :].bitcast(u16)
                    )
                    nc.vector.tensor_scalar(
                        out=at16[:, k, :], in0=ust, scalar1=0x8080,
                        scalar2=ab_sb[:, k:k + 1],
                        op0=ALU.bitwise_and, op1=ALU.bitwise_xor,
                    )
            else:
                nc.vector.memset(at16[:, :, 0:1], 0x3030)
                nc.vector.memset(bt16[:, :, 0:1], 0x3030)

            # --- DoubleRow matmuls + evacuate
            for j in range(oblocks):
                pt = psum.tile([P, psum_cols], f32, tag="ps", name=f"ps_{rep}_{j}")
                if "mm" not in skip:
                    for s in range(kpairs):
                        lhsT = at[:, 2 * s:2 * s + 2, j * P:(j + 1) * P]
                        for (c0, nw) in ntiles:
                            nc.tensor.matmul(
                                pt[:, c0:c0 + nw], lhsT=lhsT,
                                rhs=bt[:, 2 * s:2 * s + 2, c0:c0 + nw],
                                start=(s == 0), stop=(s == kpairs - 1),
                                perf_mode=DR,
                            )
                ot = outp.tile([P, n_sh], bf16, tag="out", name=f"ot_{rep}_{j}")
                ot2 = outp2.tile([P, n_sh], bf16, tag="out2", name=f"ot2_{rep}_{j}")
                if "evac" not in skip:
                    # h via ACT per-partition scale (PSUM f32 -> SBUF bf16),
                    # then g via DVE bf16 tensor_tensor (2x mode)
                    nc.scalar.activation(
                        out=ot, in_=pt[:, 0:n_sh], func=AF.Copy,
                        scale=h_sb[:, j:j + 1],
                    )
                    nc.vector.tensor_tensor(
                        out=ot2, in0=ot, in1=g_sb, op=ALU.mult,
                    )
                else:
                    nc.vector.memset(ot2[:, 0:1], 0.0)
                if "outdma" not in skip:
                    nc.gpsimd.dma_start(out=out[j * P:(j + 1) * P, :], in_=ot2)

    nc.compile()
    return nc


_NC_CACHE = {}


def _get_nc():
    if "nc" not in _NC_CACHE:
        _NC_CACHE["nc"] = build_program()
    return _NC_CACHE["nc"]


def _e4m3_normal_grid():
    import ml_dtypes

    vals = []
    for bits in range(1, 0x7F):
        f = float(np.uint8(bits).view(ml_dtypes.float8_e4m3fn))
        if np.isfinite(f) and 0.015625 <= f <= 240.0:
            vals.append(f)
    return np.array(sorted(set(vals)))


def _factorize_ell(ell):
    """Best alpha*beta ~= |ell| with both factors on the normal e4m3 grid.

    Balanced around sqrt|ell| so neither factor goes subnormal. Returns
    (alpha_f32 (>0), beta_signed_f32) with alpha * beta_signed ~= ell.
    """
    grid = _e4m3_normal_grid()
    a_ell = np.abs(ell).astype(np.float64)
    sq = np.sqrt(a_ell)
    ai = np.searchsorted(grid, sq)
    best_a = np.ones_like(a_ell)
    best_b = np.ones_like(a_ell)
    best_err = np.full_like(a_ell, np.inf)
    for off in range(-24, 25):
        idx = np.clip(ai + off, 0, len(grid) - 1)
        alpha = grid[idx]
        tgt = a_ell / alpha
        bi = np.searchsorted(grid, tgt)
        for boff in (-1, 0):
            bidx = np.clip(bi + boff, 0, len(grid) - 1)
            beta = grid[bidx]
            err = np.abs(alpha * beta - a_ell)
            take = err < best_err
            best_a = np.where(take, alpha, best_a)
            best_b = np.where(take, beta, best_b)
            best_err = np.where(take, err, best_err)
    return (
        best_a.astype(np.float32),
        (best_b * np.sign(ell)).astype(np.float32),
    )


def _make_in_maps(U_fp, V_fp, h, g, ell):
    import ml_dtypes

    FP8 = ml_dtypes.float8_e4m3fn

    U_fp = np.ascontiguousarray(np.asarray(U_fp, dtype=np.float32))
    V_fp = np.ascontiguousarray(np.asarray(V_fp, dtype=np.float32))
    h = np.asarray(h, dtype=np.float32).reshape(-1)
    g = np.asarray(g, dtype=np.float32).reshape(-1)
    ell = np.asarray(ell, dtype=np.float32).reshape(-1)

    alpha, beta_s = _factorize_ell(ell)

    # fp8 byte planes: only the sign bit of ut/vt is consumed on device.
    # Scale bytes are doubled into u16 (two fp8 lanes share the partition's
    # scale) for the packed staging op.
    ut = np.ascontiguousarray(U_fp.T).astype(FP8)            # (R, D_OUT)

    def dbl(x):  # fp8 byte -> 0xBBBB u16
        b = x.astype(FP8).view(np.uint8).astype(np.uint16)
        return (b | (b << 8)).reshape(KB, P).T.copy()        # (128, KB)

    ab16 = dbl(alpha)
    bb16 = dbl(beta_s)
    h_t = np.ascontiguousarray(h.reshape(OB, P).T)           # (128, 32)

    in_maps = []
    for c in range(NCORES):
        sl = slice(c * N_SH, (c + 1) * N_SH)
        in_maps.append({
            "ut": ut,
            "vt": np.ascontiguousarray(V_fp[sl, :].T).astype(FP8),  # (R, N_SH)
            "ab": ab16,
            "bb": bb16,
            "h": h_t,
            "g": np.ascontiguousarray(
                np.broadcast_to(g[sl].reshape(1, N_SH), (P, N_SH))
            ).astype(ml_dtypes.bfloat16),
        })
    return in_maps


def run(U_fp, V_fp, h, g, ell, trace=False):
    """Run on 8 NeuronCores; returns (M, BassKernelResults)."""
    from concourse.bass_utils import run_bass_kernel_spmd

    nc = _get_nc()
    in_maps = _make_in_maps(U_fp, V_fp, h, g, ell)
    res = run_bass_kernel_spmd(nc, in_maps, list(range(NCORES)), trace=trace)
    M = np.concatenate(
        [res.results[c]["out"].astype(np.float32) for c in range(NCORES)],
        axis=1,
    )
    return M, res


def kernel(U_fp, V_fp, h, g, ell):
    M, _ = run(U_fp, V_fp, h, g, ell, trace=False)
    return M


# revision 10
# speedup vs baseline: 1.0483x; 1.0098x over previous
"""Trainium2 Bass kernel for LittleBitLinear reconstruction (fp8 DoubleRow).

Computes M = (sign(U_fp) * ell) @ sign(V_fp)^T * g[None, :] * h[:, None]
for U_fp (4096, 1024), V_fp (11008, 1024) -> M (4096, 11008) fp32.

Strategy: shard d_in (rows of V_fp / columns of M) across 8 cores; U_fp, h,
ell replicated. Each core computes the full 4096 x 1376 column block.

Key idea: the matmul operands are pure signs scaled per contraction index r
by ell[r]. Factor |ell[r]| ~= alpha[r] * beta[r] with both factors exactly on
the fp8-e4m3 grid (error ~0.9% rms, deterministic), fold sign(ell) into
beta. Then A[r, m] = sign(U)*alpha[r] and B[r, n] = sign(V)*sign(ell)*beta[r]
are EXACT fp8 values, and the fp8 DoubleRow matmul (2x bf16 throughput,
256-deep contraction per pass, fp32 PSUM accumulation) computes
sum_r sign(U)*sign(V)*alpha*beta exactly up to fp32 accumulation. g and h are
applied exactly at PSUM evacuation (ACT per-partition scale for h, DVE
elementwise for g), so the dominant approximation is ell -> alpha*beta.

Staging needs no Sign activation: host ships U^T / V^T as raw fp8 bytes
(only the sign bit is consumed -- cast preserves it for every value incl.
+-0). The scale factor alpha[r]/beta[r] depends only on the SBUF partition,
so adjacent byte pairs share it and staging runs as uint16:
(bytes16 & 0x8080) ^ (alpha<<8|alpha) = sign*scale for two fp8 lanes at once,
hitting the DVE 16-bit fast path.
"""

import os
import sys

import numpy as np

for _p in ("/opt/trn_rl_repo",):
    if _p not in sys.path and os.path.isdir(_p):
        sys.path.insert(0, _p)

D_OUT, D_IN, R, NCORES = 4096, 11008, 1024, 8
N_SH = D_IN // NCORES  # 1376
P = 128
KB = R // P            # 8 k-blocks
KPAIR = KB // 2        # 4 double-row pairs
OB = D_OUT // P        # 32 o-blocks


def _n_tiles(n_sh, max_n=512):
    # narrow tile first: the next (j,s) LDWEIGHTS hides best under a
    # trailing full-width stream
    tiles = []
    c0 = 0
    while c0 < n_sh:
        nw = min(max_n, n_sh - c0)
        tiles.append((c0, nw))
        c0 += nw
    return tiles[::-1]


def build_program(d_out=D_OUT, n_sh=N_SH, r=R, reps=1, skip=(), max_n=512,
                  psum_cols=1536):
    """Build the per-core Bass program (SPMD: same program, different data)."""
    from contextlib import ExitStack

    import concourse.bass as bass  # noqa: F401
    import concourse.mybir as mybir
    import concourse.tile as tile
    from concourse import bacc

    f32 = mybir.dt.float32
    bf16 = mybir.dt.bfloat16
    u8 = mybir.dt.uint8
    u16 = mybir.dt.uint16
    fp8 = mybir.dt.float8e4
    AF = mybir.ActivationFunctionType
    ALU = mybir.AluOpType
    DR = mybir.MatmulPerfMode.DoubleRow

    kblocks = r // P
    kpairs = kblocks // 2
    oblocks = d_out // P
    ntiles = _n_tiles(n_sh, max_n=max_n)

    nc = bacc.Bacc(None, target_bir_lowering=False)
    ut = nc.declare_dram_parameter("ut", [r, d_out], fp8, isOutput=False)
    vt = nc.declare_dram_parameter("vt", [r, n_sh], fp8, isOutput=False)
    ab = nc.declare_dram_parameter("ab", [P, kblocks], u16, isOutput=False)
    bb = nc.declare_dram_parameter("bb", [P, kblocks], u16, isOutput=False)
    hh = nc.declare_dram_parameter("h", [P, oblocks], f32, isOutput=False)
    gg = nc.declare_dram_parameter("g", [P, n_sh], bf16, isOutput=False)
    out = nc.declare_dram_parameter("out", [d_out, n_sh], bf16, isOutput=True)

    with tile.TileContext(nc) as tc, ExitStack() as ctx:
        consts = ctx.enter_context(tc.tile_pool(name="consts", bufs=1))
        ustg = ctx.enter_context(tc.tile_pool(name="ustg", bufs=3))
        vstg = ctx.enter_context(tc.tile_pool(name="vstg", bufs=3))
        abuf = ctx.enter_context(tc.tile_pool(name="abuf", bufs=1))
        bbuf = ctx.enter_context(tc.tile_pool(name="bbuf", bufs=1))
        outp = ctx.enter_context(tc.tile_pool(name="outp", bufs=4))
        outp2 = ctx.enter_context(tc.tile_pool(name="outp2", bufs=4))
        psum = ctx.enter_context(tc.tile_pool(name="psum", bufs=2, space="PSUM"))

        # Consts arrive via the scalar ring (idle early; keeps the sync ring
        # free for ut/vt). DVE copies collapse their sync source so staging
        # TS (DVE) and evac TT (DVE) see an engine-internal dep for the
        # scale operands -- a single cross-proc wait remains (DMA / ACT).
        ab_raw = consts.tile([P, kblocks], u16)
        nc.gpsimd.dma_start(out=ab_raw, in_=ab[:, :])
        ab_sb = consts.tile([P, kblocks], u16)
        nc.vector.tensor_scalar(out=ab_sb, in0=ab_raw, scalar1=0,
                                op0=ALU.bitwise_or, scalar2=None)
        bb_raw = consts.tile([P, kblocks], u16)
        nc.gpsimd.dma_start(out=bb_raw, in_=bb[:, :])
        bb_sb = consts.tile([P, kblocks], u16)
        nc.vector.tensor_scalar(out=bb_sb, in0=bb_raw, scalar1=0,
                                op0=ALU.bitwise_or, scalar2=None)
        h_raw = consts.tile([P, oblocks], f32)
        nc.gpsimd.dma_start(out=h_raw, in_=hh[:, :])
        h_sb = consts.tile([P, oblocks], f32)
        nc.scalar.activation(out=h_sb, in_=h_raw, func=AF.Copy)
        g_raw = consts.tile([P, n_sh], bf16)
        nc.gpsimd.dma_start(out=g_raw, in_=gg[:, :])
        g_sb = consts.tile([P, n_sh], bf16)
        nc.vector.tensor_scalar(out=g_sb, in0=g_raw, scalar1=0.0,
                                op0=ALU.add, scalar2=None)

        for rep in range(reps):
            # A: [128, KB, d_out] fp8, B: [128, KB, n_sh] fp8 -- pair dim in
            # the middle so DoubleRow can slice [:, 2s:2s+2, cols].
            at = abuf.tile([P, kblocks, d_out], fp8, tag="a", name=f"at_{rep}")
            bt = bbuf.tile([P, kblocks, n_sh], fp8, tag="b", name=f"bt_{rep}")
            at16 = at.bitcast(u16)
            bt16 = bt.bitcast(u16)

            if "stage" not in skip:
                # spread input DMAs across three rings (sync/gpsimd/scalar)
                # so dispatch serialization and wire time overlap
                for k in range(kblocks):
                    # alternate input k-blocks across the sync and scalar
                    # rings so the wire runs in parallel during the ramp
                    keng = nc.sync if k % 2 == 0 else nc.scalar
                    vst = vstg.tile([P, n_sh // 2], u16, tag="vstg",
                                    name=f"vst_{rep}_{k}")
                    keng.dma_start(
                        out=vst, in_=vt[k * P:(k + 1) * P, :].bitcast(u16)
                    )
                    nc.vector.tensor_scalar(
                        out=bt16[:, k, :], in0=vst, scalar1=0x8080,
                        scalar2=bb_sb[:, k:k + 1],
                        op0=ALU.bitwise_and, op1=ALU.bitwise_xor,
                    )
                    ust = ustg.tile([P, d_out // 2], u16, tag="ustg",
                                    name=f"ust_{rep}_{k}")
                    keng.dma_start(
                        out=ust, in_=ut[k * P:(k + 1) * P, :].bitcast(u16)
                    )
                    nc.vector.tensor_scalar(
                        out=at16[:, k, :], in0=ust, scalar1=0x8080,
                        scalar2=ab_sb[:, k:k + 1],
                        op0=ALU.bitwise_and, op1=ALU.bitwise_xor,
                    )
            else:
                nc.vector.memset(at16[:, :, 0:1], 0x3030)
                nc.vector.memset(bt16[:, :, 0:1], 0x3030)

            # --- DoubleRow matmuls + evacuate
            for j in range(oblocks):
                pt = psum.tile([P, psum_cols], f32, tag="ps", name=f"ps_{rep}_{j}")
                if "mm" not in skip:
                    for s in range(kpairs):
                        lhsT = at[:, 2 * s:2 * s + 2, j * P:(j + 1) * P]
                        for (c0, nw) in ntiles:
                            nc.tensor.matmul(
                                pt[:, c0:c0 + nw], lhsT=lhsT,
                                rhs=bt[:, 2 * s:2 * s + 2, c0:c0 + nw],
                                start=(s == 0), stop=(s == kpairs - 1),
                                perf_mode=DR,
                            )
                ot = outp.tile([P, n_sh], bf16, tag="out", name=f"ot_{rep}_{j}")
                ot2 = outp2.tile([P, n_sh], bf16, tag="out2", name=f"ot2_{rep}_{j}")
                if "evac" not in skip:
                    # h via ACT per-partition scale (PSUM f32 -> SBUF bf16),
                    # then g via DVE bf16 tensor_tensor (2x mode)
                    nc.scalar.activation(
                        out=ot, in_=pt[:, 0:n_sh], func=AF.Copy,
                        scale=h_sb[:, j:j + 1],
                    )
                    nc.vector.tensor_tensor(
                        out=ot2, in0=ot, in1=g_sb, op=ALU.mult,
                    )
                else:
                    nc.vector.memset(ot2[:, 0:1], 0.0)
                if "outdma" not in skip:
                    nc.gpsimd.dma_start(out=out[j * P:(j + 1) * P, :], in_=ot2)

    nc.compile()
    return nc


_NC_CACHE = {}


def _get_nc():
    if "nc" not in _NC_CACHE:
        _NC_CACHE["nc"] = build_program()
    return _NC_CACHE["nc"]


def _e4m3_normal_grid():
    import ml_dtypes

    vals = []
    for bits in range(1, 0x7F):
        f = float(np.uint8(bits).view(ml_dtypes.float8_e4m3fn))
        if np.isfinite(f) and 0.015625 <= f <= 240.0:
            vals.append(f)
    return np.array(sorted(set(vals)))


def _factorize_ell(ell):
    """Best alpha*beta ~= |ell| with both factors on the normal e4m3 grid.

    Balanced around sqrt|ell| so neither factor goes subnormal. Returns
    (alpha_f32 (>0), beta_signed_f32) with alpha * beta_signed ~= ell.
    """
    grid = _e4m3_normal_grid()
    a_ell = np.abs(ell).astype(np.float64)
    sq = np.sqrt(a_ell)
    ai = np.searchsorted(grid, sq)
    best_a = np.ones_like(a_ell)
    best_b = np.ones_like(a_ell)
    best_err = np.full_like(a_ell, np.inf)
    for off in range(-24, 25):
        idx = np.clip(ai + off, 0, len(grid) - 1)
        alpha = grid[idx]
        tgt = a_ell / alpha
        bi = np.searchsorted(grid, tgt)
        for boff in (-1, 0):
            bidx = np.clip(bi + boff, 0, len(grid) - 1)
            beta = grid[bidx]
            err = np.abs(alpha * beta - a_ell)
            take = err < best_err
            best_a = np.where(take, alpha, best_a)
            best_b = np.where(take, beta, best_b)
            best_err = np.where(take, err, best_err)
    return (
        best_a.astype(np.float32),
        (best_b * np.sign(ell)).astype(np.float32),
    )


def _make_in_maps(U_fp, V_fp, h, g, ell):
    import ml_dtypes

    FP8 = ml_dtypes.float8_e4m3fn

    U_fp = np.ascontiguousarray(np.asarray(U_fp, dtype=np.float32))
    V_fp = np.ascontiguousarray(np.asarray(V_fp, dtype=np.float32))
    h = np.asarray(h, dtype=np.float32).reshape(-1)
    g = np.asarray(g, dtype=np.float32).reshape(-1)
    ell = np.asarray(ell, dtype=np.float32).reshape(-1)

    alpha, beta_s = _factorize_ell(ell)

    # fp8 byte planes: only the sign bit of ut/vt is consumed on device.
    # Scale bytes are doubled into u16 (two fp8 lanes share the partition's
    # scale) for the packed staging op.
    ut = np.ascontiguousarray(U_fp.T).astype(FP8)            # (R, D_OUT)

    def dbl(x):  # fp8 byte -> 0xBBBB u16
        b = x.astype(FP8).view(np.uint8).astype(np.uint16)
        return (b | (b << 8)).reshape(KB, P).T.copy()        # (128, KB)

    ab16 = dbl(alpha)
    bb16 = dbl(beta_s)
    h_t = np.ascontiguousarray(h.reshape(OB, P).T)           # (128, 32)

    in_maps = []
    for c in range(NCORES):
        sl = slice(c * N_SH, (c + 1) * N_SH)
        in_maps.append({
            "ut": ut,
            "vt": np.ascontiguousarray(V_fp[sl, :].T).astype(FP8),  # (R, N_SH)
            "ab": ab16,
            "bb": bb16,
            "h": h_t,
            "g": np.ascontiguousarray(
                np.broadcast_to(g[sl].reshape(1, N_SH), (P, N_SH))
            ).astype(ml_dtypes.bfloat16),
        })
    return in_maps


def run(U_fp, V_fp, h, g, ell, trace=False):
    """Run on 8 NeuronCores; returns (M, BassKernelResults)."""
    from concourse.bass_utils import run_bass_kernel_spmd

    nc = _get_nc()
    in_maps = _make_in_maps(U_fp, V_fp, h, g, ell)
    res = run_bass_kernel_spmd(nc, in_maps, list(range(NCORES)), trace=trace)
    M = np.concatenate(
        [res.results[c]["out"].astype(np.float32) for c in range(NCORES)],
        axis=1,
    )
    return M, res


def kernel(U_fp, V_fp, h, g, ell):
    M, _ = run(U_fp, V_fp, h, g, ell, trace=False)
    return M


# revision 12
# speedup vs baseline: 1.0711x; 1.0217x over previous
"""Trainium2 Bass kernel for LittleBitLinear reconstruction (fp8 DoubleRow).

Computes M = (sign(U_fp) * ell) @ sign(V_fp)^T * g[None, :] * h[:, None]
for U_fp (4096, 1024), V_fp (11008, 1024) -> M (4096, 11008) fp32.

Strategy: shard d_in (rows of V_fp / columns of M) across 8 cores; U_fp, h,
ell replicated. Each core computes the full 4096 x 1376 column block.

Key idea: the matmul operands are pure signs scaled per contraction index r
by ell[r]. Factor |ell[r]| ~= alpha[r] * beta[r] with both factors exactly on
the fp8-e4m3 grid (error ~0.9% rms, deterministic), fold sign(ell) into
beta. Then A[r, m] = sign(U)*alpha[r] and B[r, n] = sign(V)*sign(ell)*beta[r]
are EXACT fp8 values, and the fp8 DoubleRow matmul (2x bf16 throughput,
256-deep contraction per pass, fp32 PSUM accumulation) computes
sum_r sign(U)*sign(V)*alpha*beta exactly up to fp32 accumulation. g and h are
applied exactly at PSUM evacuation (ACT per-partition scale for h, DVE
elementwise for g), so the dominant approximation is ell -> alpha*beta.

Staging needs no Sign activation: host ships U^T / V^T as raw fp8 bytes
(only the sign bit is consumed -- cast preserves it for every value incl.
+-0). The scale factor alpha[r]/beta[r] depends only on the SBUF partition,
so adjacent byte pairs share it and staging runs as uint16:
(bytes16 & 0x8080) ^ (alpha<<8|alpha) = sign*scale for two fp8 lanes at once,
hitting the DVE 16-bit fast path.
"""

import os
import sys

import numpy as np

for _p in ("/opt/trn_rl_repo",):
    if _p not in sys.path and os.path.isdir(_p):
        sys.path.insert(0, _p)

D_OUT, D_IN, R, NCORES = 4096, 11008, 1024, 8
N_SH = D_IN // NCORES  # 1376
P = 128
KB = R // P            # 8 k-blocks
KPAIR = KB // 2        # 4 double-row pairs
OB = D_OUT // P        # 32 o-blocks


def _n_tiles(n_sh, max_n=512):
    # narrow tile first: the next (j,s) LDWEIGHTS hides best under a
    # trailing full-width stream
    tiles = []
    c0 = 0
    while c0 < n_sh:
        nw = min(max_n, n_sh - c0)
        tiles.append((c0, nw))
        c0 += nw
    return tiles[::-1]


def build_program(d_out=D_OUT, n_sh=N_SH, r=R, reps=1, skip=(), max_n=512,
                  psum_cols=1536):
    """Build the per-core Bass program (SPMD: same program, different data)."""
    from contextlib import ExitStack

    import concourse.bass as bass  # noqa: F401
    import concourse.mybir as mybir
    import concourse.tile as tile
    from concourse import bacc

    f32 = mybir.dt.float32
    bf16 = mybir.dt.bfloat16
    u8 = mybir.dt.uint8
    u16 = mybir.dt.uint16
    fp8 = mybir.dt.float8e4
    AF = mybir.ActivationFunctionType
    ALU = mybir.AluOpType
    DR = mybir.MatmulPerfMode.DoubleRow

    kblocks = r // P
    kpairs = kblocks // 2
    oblocks = d_out // P
    ntiles = _n_tiles(n_sh, max_n=max_n)

    nc = bacc.Bacc(None, target_bir_lowering=False)
    ut = nc.declare_dram_parameter("ut", [r, d_out], fp8, isOutput=False)
    vt = nc.declare_dram_parameter("vt", [r, n_sh], fp8, isOutput=False)
    ab = nc.declare_dram_parameter("ab", [P, kblocks], u16, isOutput=False)
    bb = nc.declare_dram_parameter("bb", [P, kblocks], u16, isOutput=False)
    hh = nc.declare_dram_parameter("h", [P, oblocks], f32, isOutput=False)
    gg = nc.declare_dram_parameter("g", [P, n_sh], bf16, isOutput=False)
    out = nc.declare_dram_parameter("out", [d_out, n_sh], bf16, isOutput=True)

    with tile.TileContext(nc) as tc, ExitStack() as ctx:
        consts = ctx.enter_context(tc.tile_pool(name="consts", bufs=1))
        ustg = ctx.enter_context(tc.tile_pool(name="ustg", bufs=8))
        vstg = ctx.enter_context(tc.tile_pool(name="vstg", bufs=8))
        abuf = ctx.enter_context(tc.tile_pool(name="abuf", bufs=1))
        bbuf = ctx.enter_context(tc.tile_pool(name="bbuf", bufs=1))
        outp = ctx.enter_context(tc.tile_pool(name="outp", bufs=4))
        outp2 = ctx.enter_context(tc.tile_pool(name="outp2", bufs=4))
        psum = ctx.enter_context(tc.tile_pool(name="psum", bufs=2, space="PSUM"))

        # Consts arrive via the scalar ring (idle early; keeps the sync ring
        # free for ut/vt). DVE copies collapse their sync source so staging
        # TS (DVE) and evac TT (DVE) see an engine-internal dep for the
        # scale operands -- a single cross-proc wait remains (DMA / ACT).
        ab_raw = consts.tile([P, kblocks], u16)
        nc.sync.dma_start(out=ab_raw, in_=ab[:, :])
        ab_sb = consts.tile([P, kblocks], u16)
        nc.vector.tensor_scalar(out=ab_sb, in0=ab_raw, scalar1=0,
                                op0=ALU.bitwise_or, scalar2=None)
        bb_raw = consts.tile([P, kblocks], u16)
        nc.sync.dma_start(out=bb_raw, in_=bb[:, :])
        bb_sb = consts.tile([P, kblocks], u16)
        nc.vector.tensor_scalar(out=bb_sb, in0=bb_raw, scalar1=0,
                                op0=ALU.bitwise_or, scalar2=None)
        h_raw = consts.tile([P, oblocks], f32)
        nc.gpsimd.dma_start(out=h_raw, in_=hh[:, :])
        h_sb = consts.tile([P, oblocks], f32)
        nc.scalar.activation(out=h_sb, in_=h_raw, func=AF.Copy)
        g_raw = consts.tile([P, n_sh], bf16)
        nc.gpsimd.dma_start(out=g_raw, in_=gg[:, :])
        g_sb = consts.tile([P, n_sh], bf16)
        nc.vector.tensor_scalar(out=g_sb, in0=g_raw, scalar1=0.0,
                                op0=ALU.add, scalar2=None)

        for rep in range(reps):
            # A: [128, KB, d_out] fp8, B: [128, KB, n_sh] fp8 -- pair dim in
            # the middle so DoubleRow can slice [:, 2s:2s+2, cols].
            at = abuf.tile([P, kblocks, d_out], fp8, tag="a", name=f"at_{rep}")
            bt = bbuf.tile([P, kblocks, n_sh], fp8, tag="b", name=f"bt_{rep}")
            at16 = at.bitcast(u16)
            bt16 = bt.bitcast(u16)

            if "stage" not in skip:
                # spread input DMAs across three rings (sync/gpsimd/scalar)
                # so dispatch serialization and wire time overlap
                for k in range(kblocks):
                    # alternate input k-blocks across the sync and scalar
                    # rings so the wire runs in parallel during the ramp
                    keng = nc.sync if k % 2 == 0 else nc.scalar
                    vst = vstg.tile([P, n_sh // 2], u16, tag="vstg",
                                    name=f"vst_{rep}_{k}")
                    keng.dma_start(
                        out=vst, in_=vt[k * P:(k + 1) * P, :].bitcast(u16)
                    )
                    nc.vector.tensor_scalar(
                        out=bt16[:, k, :], in0=vst, scalar1=0x8080,
                        scalar2=bb_sb[:, k:k + 1],
                        op0=ALU.bitwise_and, op1=ALU.bitwise_xor,
                    )
                    ust = ustg.tile([P, d_out // 2], u16, tag="ustg",
                                    name=f"ust_{rep}_{k}")
                    keng.dma_start(
                        out=ust, in_=ut[k * P:(k + 1) * P, :].bitcast(u16)
                    )
                    nc.vector.tensor_scalar(
                        out=at16[:, k, :], in0=ust, scalar1=0x8080,
                        scalar2=ab_sb[:, k:k + 1],
                        op0=ALU.bitwise_and, op1=ALU.bitwise_xor,
                    )
            else:
                nc.vector.memset(at16[:, :, 0:1], 0x3030)
                nc.vector.memset(bt16[:, :, 0:1], 0x3030)

            # --- DoubleRow matmuls + evacuate
            for j in range(oblocks):
                pt = psum.tile([P, psum_cols], f32, tag="ps", name=f"ps_{rep}_{j}")
                if "mm" not in skip:
                    for s in range(kpairs):
                        lhsT = at[:, 2 * s:2 * s + 2, j * P:(j + 1) * P]
                        for (c0, nw) in ntiles:
                            nc.tensor.matmul(
                                pt[:, c0:c0 + nw], lhsT=lhsT,
                                rhs=bt[:, 2 * s:2 * s + 2, c0:c0 + nw],
                                start=(s == 0), stop=(s == kpairs - 1),
                                perf_mode=DR,
                            )
                ot = outp.tile([P, n_sh], bf16, tag="out", name=f"ot_{rep}_{j}")
                ot2 = outp2.tile([P, n_sh], bf16, tag="out2", name=f"ot2_{rep}_{j}")
                if "evac" not in skip:
                    # h via ACT per-partition scale (PSUM f32 -> SBUF bf16),
                    # then g via DVE bf16 tensor_tensor (2x mode)
                    nc.scalar.activation(
                        out=ot, in_=pt[:, 0:n_sh], func=AF.Copy,
                        scale=h_sb[:, j:j + 1],
                    )
                    nc.vector.tensor_tensor(
                        out=ot2, in0=ot, in1=g_sb, op=ALU.mult,
                    )
                else:
                    nc.vector.memset(ot2[:, 0:1], 0.0)
                if "outdma" not in skip:
                    nc.gpsimd.dma_start(out=out[j * P:(j + 1) * P, :], in_=ot2)

    nc.compile()
    return nc


_NC_CACHE = {}


def _get_nc():
    if "nc" not in _NC_CACHE:
        _NC_CACHE["nc"] = build_program()
    return _NC_CACHE["nc"]


def _e4m3_normal_grid():
    import ml_dtypes

    vals = []
    for bits in range(1, 0x7F):
        f = float(np.uint8(bits).view(ml_dtypes.float8_e4m3fn))
        if np.isfinite(f) and 0.015625 <= f <= 240.0:
            vals.append(f)
    return np.array(sorted(set(vals)))


def _factorize_ell(ell):
    """Best alpha*beta ~= |ell| with both factors on the normal e4m3 grid.

    Balanced around sqrt|ell| so neither factor goes subnormal. Returns
    (alpha_f32 (>0), beta_signed_f32) with alpha * beta_signed ~= ell.
    """
    grid = _e4m3_normal_grid()
    a_ell = np.abs(ell).astype(np.float64)
    sq = np.sqrt(a_ell)
    ai = np.searchsorted(grid, sq)
    best_a = np.ones_like(a_ell)
    best_b = np.ones_like(a_ell)
    best_err = np.full_like(a_ell, np.inf)
    for off in range(-24, 25):
        idx = np.clip(ai + off, 0, len(grid) - 1)
        alpha = grid[idx]
        tgt = a_ell / alpha
        bi = np.searchsorted(grid, tgt)
        for boff in (-1, 0):
            bidx = np.clip(bi + boff, 0, len(grid) - 1)
            beta = grid[bidx]
            err = np.abs(alpha * beta - a_ell)
            take = err < best_err
            best_a = np.where(take, alpha, best_a)
            best_b = np.where(take, beta, best_b)
            best_err = np.where(take, err, best_err)
    return (
        best_a.astype(np.float32),
        (best_b * np.sign(ell)).astype(np.float32),
    )


def _make_in_maps(U_fp, V_fp, h, g, ell):
    import ml_dtypes

    FP8 = ml_dtypes.float8_e4m3fn

    U_fp = np.ascontiguousarray(np.asarray(U_fp, dtype=np.float32))
    V_fp = np.ascontiguousarray(np.asarray(V_fp, dtype=np.float32))
    h = np.asarray(h, dtype=np.float32).reshape(-1)
    g = np.asarray(g, dtype=np.float32).reshape(-1)
    ell = np.asarray(ell, dtype=np.float32).reshape(-1)

    alpha, beta_s = _factorize_ell(ell)

    # fp8 byte planes: only the sign bit of ut/vt is consumed on device.
    # Scale bytes are doubled into u16 (two fp8 lanes share the partition's
    # scale) for the packed staging op.
    ut = np.ascontiguousarray(U_fp.T).astype(FP8)            # (R, D_OUT)

    def dbl(x):  # fp8 byte -> 0xBBBB u16
        b = x.astype(FP8).view(np.uint8).astype(np.uint16)
        return (b | (b << 8)).reshape(KB, P).T.copy()        # (128, KB)

    ab16 = dbl(alpha)
    bb16 = dbl(beta_s)
    h_t = np.ascontiguousarray(h.reshape(OB, P).T)           # (128, 32)

    in_maps = []
    for c in range(NCORES):
        sl = slice(c * N_SH, (c + 1) * N_SH)
        in_maps.append({
            "ut": ut,
            "vt": np.ascontiguousarray(V_fp[sl, :].T).astype(FP8),  # (R, N_SH)
            "ab": ab16,
            "bb": bb16,
            "h": h_t,
            "g": np.ascontiguousarray(
                np.broadcast_to(g[sl].reshape(1, N_SH), (P, N_SH))
            ).astype(ml_dtypes.bfloat16),
        })
    return in_maps


def run(U_fp, V_fp, h, g, ell, trace=False):
    """Run on 8 NeuronCores; returns (M, BassKernelResults)."""
    from concourse.bass_utils import run_bass_kernel_spmd

    nc = _get_nc()
    in_maps = _make_in_maps(U_fp, V_fp, h, g, ell)
    res = run_bass_kernel_spmd(nc, in_maps, list(range(NCORES)), trace=trace)
    M = np.concatenate(
        [res.results[c]["out"].astype(np.float32) for c in range(NCORES)],
        axis=1,
    )
    return M, res


def kernel(U_fp, V_fp, h, g, ell):
    M, _ = run(U_fp, V_fp, h, g, ell, trace=False)
    return M
